# revision 15
# baseline (speedup 1.0000x reference)
"""Trainium2 Bass kernel for ChunkCausalDepthwiseConv1d.

Problem: x (16, 512, 4096) f32; per-channel depthwise convs:
  out = chunk_scale * (chunkconv_K31_same_per_256chunk(x) + chunk_b)
        + causalconv_K16(x) + causal_b

Strategy (8 NeuronCores, channel-parallel, 64 ch/core, all batches):
  The conv is cast as per-channel Toeplitz matmuls on the TensorEngine in a
  time-major (transposed) domain.  Per channel 5 stationary matrices are
  precomputed on host (scale/bias folded in):
    A_e [128,128]: within-block taps -> even 128-block of each 256-chunk
    A_o [128,128]: within-block taps -> odd 128-block
    B_e [32,32] corner: next-block taps -> even block (chunk conv lookahead)
    C_e [32,32] corner: prev-block taps -> even block (causal carry)
    C_o [32,32] corner: prev-block taps -> odd block (chunk+causal carry)
  Device pipeline per channel: DMA x natural [128=(b4,blk32), 512=(g4,t128)]
  -> 4 PE transposes -> xtm [t128, 512=(b,blk)] -> 5 matmuls (fp32r) into
  psum_e/psum_o [128,256] -> ACT copy+bias -> 4 PE transposes back ->
  onat [(b,u) 128, t256] -> DMA out (contiguous 1KiB runs).
"""

import numpy as np

B, C, T = 16, 512, 4096
NCORES = 8
NCH = C // NCORES          # 64 channels per core
NBLK = T // 128            # 32 blocks of 128 per batch
NU = T // 256              # 16 chunks per batch
PACKW = 456                # cols: A_e 128 | A_o 128 | B 128 | C_e 32 | C_o 32 | biases 2 | pad


def _pack_weights(causal_w, causal_b, chunk_w, chunk_b, conv_scale):
    """Build (C, 128, PACKW) f32 stationary operands, scale/bias folded in."""
    w1 = np.asarray(causal_w, np.float32)[:, 0, :]     # (C,16)
    b1 = np.asarray(causal_b, np.float32)              # (C,)
    w2 = np.asarray(chunk_w, np.float32)[:, 0, :]      # (C,31)
    b2 = np.asarray(chunk_b, np.float32)               # (C,)
    cs = np.asarray(conv_scale, np.float32)            # (2,C,31)

    scale = np.ones((C, 256), np.float32)
    scale[:, :31] += cs[0]
    scale[:, 225:] += cs[1]

    k = np.arange(128)[:, None]
    m = np.arange(128)[None, :]
    d = k - m + 15
    band2 = (d >= 0) & (d <= 30)
    band1 = (d >= 0) & (d <= 15)
    d2 = np.clip(d, 0, 30)
    d1 = np.clip(d, 0, 15)

    w2d = w2[:, d2] * band2            # (C,128,128)
    w1d = w1[:, d1] * band1
    A_e = w2d * scale[:, None, :128] + w1d
    A_o = w2d * scale[:, None, 128:] + w1d

    kc = np.arange(32)[:, None]
    mc = np.arange(32)[None, :]
    kB = np.arange(32)[:, None]
    mB_ = np.arange(128)[None, :]
    dB = kB + 143 - mB_                # B (next-block -> even): w2 taps, k<15, m>=113
    mB = (dB >= 0) & (dB <= 30)
    Bw = w2[:, np.clip(dB, 0, 30)] * mB            # (C,32,128)
    dC = kc - mc - 17                  # C corners: taps 0..15 / 0..14
    mC1 = (dC >= 0) & (dC <= 15)
    Ce_t = w1[:, np.clip(dC, 0, 15)] * mC1
    Co_t = (w2[:, np.clip(dC, 0, 30)] + w1[:, np.clip(dC, 0, 15)]) * mC1

    pack = np.zeros((C, 128, PACKW), np.float32)
    pack[:, :, 0:128] = A_e
    pack[:, :, 128:256] = A_o
    pack[:, 0:32, 256:384] = Bw
    pack[:, 96:128, 384:416] = Ce_t
    pack[:, 96:128, 416:448] = Co_t
    pack[:, :, 448] = scale[:, :128] * b2[:, None] + b1[:, None]   # bias_e
    pack[:, :, 449] = scale[:, 128:] * b2[:, None] + b1[:, None]   # bias_o
    return pack


def build_nc(nch=NCH, enable_asserts=False):
    """Build the per-core Bass program (same NEFF for all cores)."""
    import concourse.bacc as bacc
    import concourse.mybir as mybir
    import concourse.tile as tile

    fp32 = mybir.dt.float32
    fp32r = mybir.dt.float32r
    COPY = mybir.ActivationFunctionType.Identity

    nc = bacc.Bacc("TRN2", target_bir_lowering=False, debug=False,
                   enable_asserts=enable_asserts)

    x_d = nc.dram_tensor("x", [B, nch, T], fp32, kind="ExternalInput").ap()
    w_d = nc.dram_tensor("wpack", [nch, 128, PACKW], fp32r, kind="ExternalInput").ap()
    id_d = nc.dram_tensor("ident", [128, 128], fp32, kind="ExternalInput").ap()
    o_d = nc.dram_tensor("out", [B, nch, T], fp32, kind="ExternalOutput").ap()

    # DRAM views
    # input: per channel [bs4, blk32, g4, t128] -> sbuf [128, 512]
    x_v = x_d.rearrange("(g bs) c (blk t) -> c bs blk g t", g=4, t=128)
    # output: per (channel, half) [b8, u16, s256]
    o_v = o_d.rearrange("(h b) c (u s) -> c h b u s", h=2, s=256)
    # weights: chunks of channels
    WCH = min(8, nch)  # channels per weight DMA
    w_v = w_d.rearrange("(cc c) p w -> cc p c w", cc=nch // WCH)

    with tile.TileContext(nc) as tc:
        with (
            tc.tile_pool(name="wbuf", bufs=1) as wbuf_pool,
            tc.tile_pool(name="ident", bufs=1) as id_pool,
            tc.tile_pool(name="xnat", bufs=3) as xnat_pool,
            tc.tile_pool(name="xtm", bufs=2) as xtm_pool,
            tc.tile_pool(name="otm", bufs=3) as otm_pool,
            tc.tile_pool(name="onat", bufs=3) as onat_pool,
            tc.tile_pool(name="ps_it", bufs=2, space="PSUM") as psit_pool,
            tc.tile_pool(name="ps_conv", bufs=3, space="PSUM") as psconv_pool,
            tc.tile_pool(name="ps_ot", bufs=2, space="PSUM") as psot_pool,
        ):
            wbuf = wbuf_pool.tile([128, nch, PACKW], fp32r)
            ident = id_pool.tile([128, 128], fp32)
            ztile = id_pool.tile([128, 16], fp32r, tag="ztile")
            nc.vector.memset(ztile[:].bitcast(fp32), 0.0)
            nc.sync.dma_start(ident[:], id_d)
            for i in range(nch // WCH):
                nc.sync.dma_start(wbuf[:, i * WCH:(i + 1) * WCH, :], w_v[i])

            for c in range(nch):
                # --- load x natural, transpose to time-major ---
                # xtm layout: 34 cols per batch = [1 zero | 32 blocks | 1 pad];
                # col b*34 + 1 + blk.  The zero col feeds the causal carry of
                # each batch's first chunk.
                xnat = xnat_pool.tile([128, 512], fp32)
                xtm = xtm_pool.tile([128, 544], fp32r)
                xtm_b = xtm[:].rearrange("p (b e) -> p b e", e=34)
                for g in range(4):
                    nc.sync.dma_start(
                        xnat[:, g * 128:(g + 1) * 128], x_v[c, :, :, g])
                    ps = psit_pool.tile([128, 128], fp32, tag="ps_it")
                    nc.tensor.transpose(ps[:], xnat[:, g * 128:(g + 1) * 128],
                                        ident[:])
                    nc.vector.tensor_copy(
                        xtm_b[:, 4 * g:4 * g + 4, 1:33], ps[:])
                nc.vector.tensor_copy(xtm_b[:, :, 0], ztile[:])

                # --- conv matmuls (fp32r) ---
                # col b*34 + 2*u2 + two; even blk 2u -> (u2=u, two=1),
                # odd blk 2u+1 -> (u2=u+1, two=0), zero col -> (0, 0).
                xr_r = xtm[:].rearrange("p (b u2 two) -> p b u2 two", b=16, two=2)
                rhs_even = xr_r[:, :, 0:16, 1]
                rhs_odd = xr_r[:, :, 1:17, 0]
                wA_e = wbuf[:, c, 0:128]
                wA_o = wbuf[:, c, 128:256]
                wB = wbuf[:, c, 256:384]
                wCe = wbuf[:, c, 384:416]
                wCo = wbuf[:, c, 416:448]

                ps_e = psconv_pool.tile([128, 256], fp32, tag="ps_conv")
                nc.tensor.matmul(ps_e[:], wA_e, rhs_even,
                                 start=True, stop=False, skip_group_check=True)
                nc.tensor.matmul(ps_e[:], wB, rhs_odd,
                                 start=False, stop=False, skip_group_check=True)
                nc.tensor.matmul(ps_e[0:32, :], wCe, xr_r[:, :, 0:16, 0],
                                 start=False, stop=True, skip_group_check=True)

                ps_o = psconv_pool.tile([128, 256], fp32, tag="ps_conv")
                nc.tensor.matmul(ps_o[:], wA_o, rhs_odd,
                                 start=True, stop=False, skip_group_check=True)
                nc.tensor.matmul(ps_o[0:32, :], wCo, rhs_even,
                                 start=False, stop=True, skip_group_check=True)

                # --- evacuate with bias ---
                otm_e = otm_pool.tile([128, 256], fp32, tag="otm")
                otm_o = otm_pool.tile([128, 256], fp32, tag="otm")
                nc.scalar.activation(otm_e[:], ps_e[:], COPY,
                                     bias=wbuf[:, c, 448:449].bitcast(fp32))
                nc.scalar.activation(otm_o[:], ps_o[:], COPY,
                                     bias=wbuf[:, c, 449:450].bitcast(fp32))

                # --- transpose back to natural, store ---
                for h in range(2):
                    onat = onat_pool.tile([128, 256], fp32, tag="onat")
                    for par, otm in ((0, otm_e), (1, otm_o)):
                        ps = psot_pool.tile([128, 128], fp32, tag="ps_ot")
                        nc.tensor.transpose(
                            ps[:], otm[:, h * 128:(h + 1) * 128], ident[:])
                        nc.vector.tensor_copy(
                            onat[:, par * 128:(par + 1) * 128], ps[:])
                    nc.sync.dma_start(o_v[c, h], onat[:])

    nc.compile()
    return nc


def kernel(x, causal_w, causal_b, chunk_w, chunk_b, conv_scale, chunk_size):
    from concourse.bass_utils import run_bass_kernel_spmd

    assert int(chunk_size) == 256
    x = np.ascontiguousarray(np.asarray(x, np.float32))
    pack = _pack_weights(causal_w, causal_b, chunk_w, chunk_b, conv_scale)
    ident = np.eye(128, dtype=np.float32)

    nc = build_nc()
    core_ids = list(range(NCORES))
    in_maps = []
    for i in core_ids:
        in_maps.append({
            "x": np.ascontiguousarray(x[:, i * NCH:(i + 1) * NCH, :]),
            "wpack": np.ascontiguousarray(pack[i * NCH:(i + 1) * NCH]),
            "ident": ident,
        })
    res = run_bass_kernel_spmd(nc, in_maps, core_ids)
    out = np.empty((B, C, T), np.float32)
    for i in core_ids:
        out[:, i * NCH:(i + 1) * NCH, :] = res.results[i]["out"]
    return out
